# revision 5
# baseline (speedup 1.0000x reference)
"""Trainium2 Bass kernel for nn_AccumulateLoss (pose-fusion squared-error loss).

reference math (CONTINLEN=5 -> 10 pairs, 10 triples (i,k,j), batch B=262144):
  fuse_rota  = R[ik] @ R[kj]            (batched 3x3 matmul)
  fuse_trans = R[ik] @ t[ik] + t[kj]
  loss = 50 * sum((fuse_rota - R[ij])^2) + sum((fuse_trans - t[ij])^2)

Strategy: pure data parallel over the batch dim across 8 NeuronCores
(32768 batch elems/core). Each core holds all 10 pairs for its batch
shard in SBUF ([128 partitions x 256 free slots] per pair), computes the
batched 3x3 products as elementwise vector ops with broadcast access
patterns, squares+reduces on the scalar engine (activation Square with
accum_out), and returns [128, 40] per-partition partial sums. The final
reduction (partitions, cores, 50x weighting) happens on host in float64.
"""
import numpy as np

# ---- problem constants (hardcoded; kernel.py must be self-contained) ----
N_CORES = 8
NPAIR = 10
B_FULL = 262144
B_CORE = B_FULL // N_CORES       # 32768
P = 128                          # SBUF partitions
F_TOT = B_CORE // P              # 256 free batch slots per partition
NCHUNK = 2
W = F_TOT // NCHUNK              # 128 batch slots per chunk
BETA = 50.0


def _triple_indices(n=5):
    pair_id = {}
    p = 0
    for a in range(n):
        for b in range(a + 1, n):
            pair_id[(a, b)] = p
            p += 1
    i1, i2, i12 = [], [], []
    for i in range(n):
        for j in range(i + 2, n):
            for k in range(i + 1, j):
                i1.append(pair_id[(i, k)])
                i2.append(pair_id[(k, j)])
                i12.append(pair_id[(i, j)])
    return i1, i2, i12


I1, I2, I12 = _triple_indices()
T = len(I1)                      # 10 triples
NCOL = 2 * T * NCHUNK            # 40 output columns


_NC_CACHE = {}


def _build_nc(repeat=1):
    import contextlib
    import concourse.tile as tile
    from concourse import bacc, mybir

    nc = bacc.Bacc("TRN2", target_bir_lowering=False, debug=False,
                   num_devices=N_CORES)
    r_ext = nc.declare_dram_parameter(
        "rotas", [NPAIR, B_CORE, 3, 3], mybir.dt.float32, isOutput=False)
    t_ext = nc.declare_dram_parameter(
        "transs", [NPAIR, B_CORE, 3], mybir.dt.float32, isOutput=False)
    out_ext = nc.declare_dram_parameter(
        "out", [P, NCOL], mybir.dt.float32, isOutput=True)

    f32 = mybir.dt.float32
    mult = mybir.AluOpType.mult
    add = mybir.AluOpType.add
    sub = mybir.AluOpType.subtract
    SQ = mybir.ActivationFunctionType.Square

    # DRAM views: batch b = p*F_TOT + f  (partition-major)
    r_view = r_ext.ap().rearrange("q (p f) i j -> q p (f i j)", p=P)
    t_view = t_ext.ap().rearrange("q (p f) i -> q p (f i)", p=P)

    with tile.TileContext(nc) as tc:
        with tc.tile_pool(name="data", bufs=2) as data_pool, \
             tc.tile_pool(name="work", bufs=2) as work_pool, \
             tc.tile_pool(name="acc", bufs=1) as acc_pool:
            loss = acc_pool.tile([P, NCOL], f32)

            def emit_chunk(c):
                Rbuf = data_pool.tile([P, NPAIR * W * 9], f32, tag="Rbuf")
                Tbuf = data_pool.tile([P, NPAIR * W * 3], f32, tag="Tbuf")
                for q in range(NPAIR):
                    nc.sync.dma_start(
                        Rbuf[:, q * W * 9:(q + 1) * W * 9],
                        r_view[q, :, c * W * 9:(c + 1) * W * 9])
                    nc.sync.dma_start(
                        Tbuf[:, q * W * 3:(q + 1) * W * 3],
                        t_view[q, :, c * W * 3:(c + 1) * W * 3])

                R5 = Rbuf[:].rearrange("p (q w i j) -> p q w i j",
                                       q=NPAIR, w=W, i=3, j=3)
                T4 = Tbuf[:].rearrange("p (q w i) -> p q w i",
                                       q=NPAIR, w=W, i=3)

                for t in range(T):
                    R1 = R5[:, I1[t]]      # [P, w, i, j]
                    R2 = R5[:, I2[t]]
                    R12 = R5[:, I12[t]]

                    p0 = work_pool.tile([P, W * 9], f32, tag="p0")
                    p1 = work_pool.tile([P, W * 9], f32, tag="p1")
                    p2 = work_pool.tile([P, W * 9], f32, tag="p2")
                    pk = [p0, p1, p2]
                    for k in range(3):
                        in0 = R1[:, :, :, k].unsqueeze(3).broadcast_to(
                            [P, W, 3, 3])
                        in1 = R2[:, :, k, :].unsqueeze(2).broadcast_to(
                            [P, W, 3, 3])
                        out = pk[k][:].rearrange("p (w i j) -> p w i j",
                                                 w=W, i=3, j=3)
                        nc.vector.tensor_tensor(out, in0, in1, mult)
                    nc.vector.tensor_tensor(p0[:], p0[:], p1[:], add)
                    nc.vector.tensor_tensor(p0[:], p0[:], p2[:], add)
                    nc.vector.tensor_tensor(
                        p0[:], p0[:], R12.rearrange("p w i j -> p (w i j)"),
                        sub)
                    col = c * 2 * T + t
                    nc.scalar.activation(p0[:], p0[:], SQ,
                                         accum_out=loss[:, col:col + 1])

                    # trans part: diff = R1 @ t1 + t2 - t12
                    t1 = T4[:, I1[t]]      # [P, w, i]
                    t2 = T4[:, I2[t]]
                    t12 = T4[:, I12[t]]
                    d = work_pool.tile([P, W * 3], f32, tag="d")
                    q0 = work_pool.tile([P, W * 3], f32, tag="q0")
                    q1 = work_pool.tile([P, W * 3], f32, tag="q1")
                    q2 = work_pool.tile([P, W * 3], f32, tag="q2")
                    nc.vector.tensor_tensor(
                        d[:].rearrange("p (w i) -> p w i", w=W, i=3),
                        t2, t12, sub)
                    qk = [q0, q1, q2]
                    for j in range(3):
                        in0 = R1[:, :, :, j]
                        in1 = t1[:, :, j].unsqueeze(2).broadcast_to([P, W, 3])
                        out = qk[j][:].rearrange("p (w i) -> p w i", w=W, i=3)
                        nc.vector.tensor_tensor(out, in0, in1, mult)
                    nc.vector.tensor_tensor(q0[:], q0[:], q1[:], add)
                    nc.vector.tensor_tensor(q2[:], q2[:], d[:], add)
                    nc.vector.tensor_tensor(q0[:], q0[:], q2[:], add)
                    col = c * 2 * T + T + t
                    nc.scalar.activation(q0[:], q0[:], SQ,
                                         accum_out=loss[:, col:col + 1])

            if repeat > 1:
                with tc.For_i(0, repeat, 1):
                    for c in range(NCHUNK):
                        emit_chunk(c)
            else:
                for c in range(NCHUNK):
                    emit_chunk(c)

            nc.sync.dma_start(out_ext.ap(), loss[:])

    nc.compile()
    return nc


def _get_nc(repeat=1):
    key = ("nc", repeat)
    if key not in _NC_CACHE:
        _NC_CACHE[key] = _build_nc(repeat)
    return _NC_CACHE[key]


def run_on_cores(rotas, transs, trace=False):
    """Shard, run SPMD on 8 cores, return (per-core col sums [8,P,NCOL], results obj)."""
    from concourse.bass_utils import run_bass_kernel_spmd

    nc = _get_nc()
    in_maps = []
    for c in range(N_CORES):
        sl = slice(c * B_CORE, (c + 1) * B_CORE)
        in_maps.append({
            "rotas": np.ascontiguousarray(rotas[:, sl], dtype=np.float32),
            "transs": np.ascontiguousarray(transs[:, sl], dtype=np.float32),
        })
    res = run_bass_kernel_spmd(nc, in_maps, core_ids=list(range(N_CORES)),
                               trace=trace)
    cols = np.stack([np.asarray(res.results[i]["out"])
                     for i in range(N_CORES)])
    return cols, res


def _reduce_cols(cols):
    """cols: [n_cores, P, NCOL] -> scalar loss (float64 host reduction)."""
    cols = cols.astype(np.float64)
    v = cols.reshape(-1, NCHUNK, 2, T)  # [cores*P, chunk, rota/trans, triple]
    rota = v[:, :, 0, :].sum()
    trans = v[:, :, 1, :].sum()
    return rota * BETA + trans


def kernel(rotas, transs):
    rotas = np.asarray(rotas)
    transs = np.asarray(transs)
    cols, _ = run_on_cores(rotas, transs, trace=False)
    return np.array([_reduce_cols(cols)], dtype=np.float32)


# revision 6
# speedup vs baseline: 1.4317x; 1.4317x over previous
"""Trainium2 Bass kernel for nn_AccumulateLoss (pose-fusion squared-error loss).

reference math (CONTINLEN=5 -> 10 pairs, 10 triples (i,k,j), batch B=262144):
  fuse_rota  = R[ik] @ R[kj]            (batched 3x3 matmul)
  fuse_trans = R[ik] @ t[ik] + t[kj]
  loss = 50 * sum((fuse_rota - R[ij])^2) + sum((fuse_trans - t[ij])^2)

Strategy: pure data parallel over the batch dim across 8 NeuronCores
(32768 batch elems/core). Each core holds all 10 pairs for its batch
shard in SBUF ([128 partitions x free slots] per pair), computes the
batched 3x3 products as elementwise ops with broadcast access patterns.
Work is split across engines: VectorE (DVE) does most rota products/sums,
GPSIMD (Pool) does the trans part (grouped by shared R1@t1 subexpression
across triples with common (i,k)) plus some rota products, ScalarE does
square+reduce via activation(Square, accum_out). Per-core [128, NCOL]
partial sums are reduced on host in float64.

Triples are grouped by (i,k): within a group, R1=R[ik] and t1=t[ik] are
fixed while the j-indexed pairs (k,j) and (i,j) occupy consecutive pair
ids -> contiguous SBUF slices.
"""
import numpy as np

# ---- problem constants (hardcoded; kernel.py must be self-contained) ----
N_CORES = 8
CONTINLEN = 5
NPAIR = 10
B_FULL = 262144
B_CORE = B_FULL // N_CORES       # 32768
P = 128                          # SBUF partitions
F_TOT = B_CORE // P              # 256 free batch slots per partition
BETA = 50.0

# ---- tunables ----
NCHUNK = 2
W = F_TOT // NCHUNK              # batch slots per chunk
N_POOL_MULT = 8                  # rota k=2 products sent to GPSIMD (0..10)
POOL_TRANS_D = True              # d/diff ops on GPSIMD (else DVE)


def _pair_id():
    pid = {}
    p = 0
    for a in range(CONTINLEN):
        for b in range(a + 1, CONTINLEN):
            pid[(a, b)] = p
            p += 1
    return pid


_PID = _pair_id()

# groups of triples (i,k,j) sharing (i,k); j in [k+1, CONTINLEN)
# each: (i1 pair, first i2 pair, first i12 pair, group size)
GROUPS = []
for _i in range(CONTINLEN):
    for _k in range(_i + 1, CONTINLEN - 1):
        GROUPS.append((_PID[(_i, _k)], _PID[(_k, _k + 1)],
                       _PID[(_i, _k + 1)], CONTINLEN - 1 - _k))
NGRP = len(GROUPS)               # 6
# flat triple list in group order (for rota): (i1, i2, i12)
TRIPLES = []
for _g, (_q1, _q2, _q12, _G) in enumerate(GROUPS):
    for _j in range(_G):
        TRIPLES.append((_q1, _q2 + _j, _q12 + _j))
T = len(TRIPLES)                 # 10
NCOL_C = T + NGRP                # per-chunk loss columns (rota + trans)
NCOL = NCOL_C * NCHUNK

_NC_CACHE = {}


def _build_nc(repeat=1):
    import contextlib
    import concourse.tile as tile
    from concourse import bacc, mybir

    nc = bacc.Bacc("TRN2", target_bir_lowering=False, debug=False,
                   num_devices=N_CORES)
    r_ext = nc.declare_dram_parameter(
        "rotas", [NPAIR, B_CORE, 3, 3], mybir.dt.float32, isOutput=False)
    t_ext = nc.declare_dram_parameter(
        "transs", [NPAIR, B_CORE, 3], mybir.dt.float32, isOutput=False)
    out_ext = nc.declare_dram_parameter(
        "out", [P, NCOL], mybir.dt.float32, isOutput=True)

    f32 = mybir.dt.float32
    mult = mybir.AluOpType.mult
    add = mybir.AluOpType.add
    sub = mybir.AluOpType.subtract
    SQ = mybir.ActivationFunctionType.Square

    # DRAM views: batch b = p*F_TOT + f  (partition-major)
    r_view = r_ext.ap().rearrange("q (p f) i j -> q p (f i j)", p=P)
    t_view = t_ext.ap().rearrange("q (p f) i -> q p (f i)", p=P)

    with tile.TileContext(nc) as tc:
        with tc.tile_pool(name="data", bufs=2) as data_pool, \
             tc.tile_pool(name="work", bufs=2) as work_pool, \
             tc.tile_pool(name="acc", bufs=1) as acc_pool:
            loss = acc_pool.tile([P, NCOL], f32)

            def emit_chunk(c):
                Rbuf = data_pool.tile([P, NPAIR * W * 9], f32, tag="Rbuf")
                Tbuf = data_pool.tile([P, NPAIR * W * 3], f32, tag="Tbuf")
                for q in range(NPAIR):
                    nc.sync.dma_start(
                        Rbuf[:, q * W * 9:(q + 1) * W * 9],
                        r_view[q, :, c * W * 9:(c + 1) * W * 9])
                    nc.sync.dma_start(
                        Tbuf[:, q * W * 3:(q + 1) * W * 3],
                        t_view[q, :, c * W * 3:(c + 1) * W * 3])

                R5 = Rbuf[:].rearrange("p (q w i j) -> p q w i j",
                                       q=NPAIR, w=W, i=3, j=3)
                T4 = Tbuf[:].rearrange("p (q w i) -> p q w i",
                                       q=NPAIR, w=W, i=3)

                # ---------------- rota: per triple ----------------
                for t, (i1, i2, i12) in enumerate(TRIPLES):
                    R1 = R5[:, i1]      # [P, w, i, j]
                    R2 = R5[:, i2]
                    R12 = R5[:, i12]
                    p0 = work_pool.tile([P, W * 9], f32, tag="p0")
                    p1 = work_pool.tile([P, W * 9], f32, tag="p1")
                    p2 = work_pool.tile([P, W * 9], f32, tag="p2")
                    pk = [p0, p1, p2]
                    for k in range(3):
                        in0 = R1[:, :, :, k].unsqueeze(3).broadcast_to(
                            [P, W, 3, 3])
                        in1 = R2[:, :, k, :].unsqueeze(2).broadcast_to(
                            [P, W, 3, 3])
                        out = pk[k][:].rearrange("p (w i j) -> p w i j",
                                                 w=W, i=3, j=3)
                        eng = nc.gpsimd if (k == 2 and t < N_POOL_MULT) \
                            else nc.vector
                        eng.tensor_tensor(out, in0, in1, mult)
                    nc.vector.tensor_tensor(p0[:], p0[:], p1[:], add)
                    nc.vector.tensor_tensor(p0[:], p0[:], p2[:], add)
                    nc.vector.tensor_tensor(
                        p0[:], p0[:], R12.rearrange("p w i j -> p (w i j)"),
                        sub)
                    col = c * NCOL_C + t
                    nc.scalar.activation(p0[:], p0[:], SQ,
                                         accum_out=loss[:, col:col + 1])

                # ------------- trans: per group on GPSIMD -------------
                for g, (q1, q2_0, q12_0, G) in enumerate(GROUPS):
                    R1 = R5[:, q1]
                    t1 = T4[:, q1]                      # [P, w, i]
                    # v = R1 @ t1   [P, w, i]
                    v0 = work_pool.tile([P, W * 3], f32, tag="v0")
                    v1 = work_pool.tile([P, W * 3], f32, tag="v1")
                    v2 = work_pool.tile([P, W * 3], f32, tag="v2")
                    vk = [v0, v1, v2]
                    for j in range(3):
                        in0 = R1[:, :, :, j]
                        in1 = t1[:, :, j].unsqueeze(2).broadcast_to([P, W, 3])
                        out = vk[j][:].rearrange("p (w i) -> p w i", w=W, i=3)
                        nc.gpsimd.tensor_tensor(out, in0, in1, mult)
                    nc.gpsimd.tensor_tensor(v0[:], v0[:], v1[:], add)
                    nc.gpsimd.tensor_tensor(v0[:], v0[:], v2[:], add)
                    # d = t2 - t12 over the whole group  [P, g, w, i]
                    dg = work_pool.tile([P, G * W * 3], f32, tag="dg")
                    d4 = dg[:].rearrange("p (g w i) -> p g w i", g=G, w=W, i=3)
                    t2g = T4[:, q2_0:q2_0 + G]
                    t12g = T4[:, q12_0:q12_0 + G]
                    eng = nc.gpsimd if POOL_TRANS_D else nc.vector
                    eng.tensor_tensor(d4, t2g, t12g, sub)
                    # diff = v (broadcast over g) + d
                    vbc = v0[:].rearrange("p (w i) -> p w i", w=W, i=3) \
                        .unsqueeze(1).broadcast_to([P, G, W, 3])
                    eng.tensor_tensor(d4, vbc, d4, add)
                    col = c * NCOL_C + T + g
                    nc.scalar.activation(dg[:], dg[:], SQ,
                                         accum_out=loss[:, col:col + 1])

            if repeat > 1:
                with tc.For_i(0, repeat, 1):
                    for c in range(NCHUNK):
                        emit_chunk(c)
            else:
                for c in range(NCHUNK):
                    emit_chunk(c)

            nc.sync.dma_start(out_ext.ap(), loss[:])

    nc.compile()
    return nc


def _get_nc(repeat=1):
    key = ("nc", repeat)
    if key not in _NC_CACHE:
        _NC_CACHE[key] = _build_nc(repeat)
    return _NC_CACHE[key]


def make_in_maps(rotas, transs):
    in_maps = []
    for c in range(N_CORES):
        sl = slice(c * B_CORE, (c + 1) * B_CORE)
        in_maps.append({
            "rotas": np.ascontiguousarray(rotas[:, sl], dtype=np.float32),
            "transs": np.ascontiguousarray(transs[:, sl], dtype=np.float32),
        })
    return in_maps


def run_on_cores(rotas, transs):
    from concourse.bass_utils import run_bass_kernel_spmd

    nc = _get_nc()
    in_maps = make_in_maps(rotas, transs)
    res = run_bass_kernel_spmd(nc, in_maps, core_ids=list(range(N_CORES)))
    cols = np.stack([np.asarray(res.results[i]["out"])
                     for i in range(N_CORES)])
    return cols, res


def _reduce_cols(cols):
    """cols: [n_cores, P, NCOL] -> scalar loss (float64 host reduction)."""
    v = cols.astype(np.float64).reshape(-1, NCHUNK, NCOL_C)
    rota = v[:, :, :T].sum()
    trans = v[:, :, T:].sum()
    return rota * BETA + trans


def kernel(rotas, transs):
    rotas = np.asarray(rotas)
    transs = np.asarray(transs)
    cols, _ = run_on_cores(rotas, transs)
    return np.array([_reduce_cols(cols)], dtype=np.float32)
